# revision 61
# baseline (speedup 1.0000x reference)
"""Causal self-attention (RMSNorm-QK + RoPE) Trainium2 Bass kernel.

Problem: B=2, T=2048, C=1024, H=16 heads, D=64.
Sharding: 8 cores = 2 (batch) x 4 (head groups of 4 heads).
Each core computes q/k/v projections for its 4 heads, attention, and a
partial output projection (column-parallel over heads); the host sums the
4 partials per batch and transposes.

All matmuls run in bf16 (inputs rounded on host) with f32 PSUM accumulation.
bf16 halves HBM traffic vs f32r, draws less PE power (less HAM throttle),
and runs full-rate at any free size. Avoid f32r DMA loads entirely: the
f32r-rounding DMA pass truncates mantissas over a wider SBUF region than
its own tile and corrupts bf16 neighbours.

Per-core layouts:
  projection chunks [128, 512]: row 32h+i = head h, rope-half dim i
  q1/q2/k1/k2[n]  [128, 512] bf16 : rope outputs per t-block, kept in the
      32h+i row layout; scores contract rc1+rc2 with two K=32 matmuls per
      head at PE row-group 32h (heads of a pair run concurrently).
  v_r[s]          [128, 4, 65] bf16 : key-chunk s, head h at [:, h, 0:64],
      ones column at [:, h, 64] (softmax denominator trick)
  S (tag SS)      [128, 2, 512] f32 PSUM : scores for one head pair
  yT32[c]         [128, 2048] f32 : heads (2c, 2c+1) attention numerator
  yT_bf[c]        [128, 2048] bf16 : normalized (divided by denominator)
Output: outT [1024, 2048] bf16 = (partial out).T per core; host sums.

Pipelining: block n's RMS+rope tensor/vector work is deferred until block
n+1's projection matmuls are issued (the scalar Ln/Exp latency hides under
them); block j's normalize+out-projection matmuls are spread as fillers
into block j+1's attention stream so the tensor engine never idles long
enough to trip the HAM half-clock throttle.
"""

import sys

for _p in ("/opt/trn_rl_repo",):
    if _p not in sys.path:
        sys.path.append(_p)

import numpy as np
import ml_dtypes

B, T, C = 2, 2048, 1024
H_TOT, D = 16, 64
HPC = 4               # heads per core
N_CORES = 8
P = 128               # partitions
NB = 4                # t-blocks of 512
TB = 512              # t-block size
KCH = 8               # C / 128 contraction chunks
RMS_EPS = 1.1920928955078125e-07
ROPE_BASE = 10000.0

_CACHE = {}


def _build_consts():
    """Host-side constant tensors shared by all cores."""
    inv_freq = (1.0 / (ROPE_BASE ** (np.arange(0, D, 2, dtype=np.float32) / np.float32(D)))).astype(np.float32)
    pos = np.arange(T, dtype=np.float32)
    freqs = np.outer(pos, inv_freq).astype(np.float32)      # [T, 32]
    cos = np.cos(freqs).astype(np.float32)                  # [T, 32]
    sin = np.sin(freqs).astype(np.float32)
    cosr = np.ascontiguousarray(np.tile(cos.T, (HPC, 1))).astype(ml_dtypes.bfloat16)
    sinr = np.ascontiguousarray(np.tile(sin.T, (HPC, 1))).astype(ml_dtypes.bfloat16)
    # ind32 [128, 4]: per-32-row-group summing matrix (lhsT for RMS sums)
    ind32 = np.zeros((P, HPC), dtype=np.float32)
    for p_ in range(P):
        ind32[p_, p_ // 32] = 1.0
    # bc32 [4, 128]: broadcast inv (4 heads) to 32-row groups (lhsT)
    bc32 = np.zeros((HPC, P), dtype=np.float32)
    for p_ in range(P):
        bc32[p_ // 32, p_] = 1.0
    # selpair4 [4, 256]: pair c: out row m <- den row (2c + m//64)
    selpair4 = np.zeros((HPC, 2 * P), dtype=np.float32)
    for c in range(2):
        for m in range(P):
            selpair4[2 * c + m // 64, 128 * c + m] = 1.0
    # causal triangle mask [128, 2, 128] bf16 (same triangle both halves):
    # keep element (p, :, i) iff i >= p
    tri = (np.arange(P)[None, :] >= np.arange(P)[:, None]).astype(np.float32)
    maskt = np.ascontiguousarray(
        np.broadcast_to(tri[:, None, :], (P, 2, P))).astype(ml_dtypes.bfloat16)
    bf = ml_dtypes.bfloat16
    return dict(cosr=cosr, sinr=sinr, ind32=ind32.astype(bf),
                bc32=bc32.astype(bf), selpair4=selpair4.astype(bf),
                maskt=maskt)


def _build_module():
    import concourse.bacc as bacc
    import concourse.mybir as mybir
    import concourse.tile as tile

    f32 = mybir.dt.float32
    bf16 = mybir.dt.bfloat16
    Exp = mybir.ActivationFunctionType.Exp
    Ln = mybir.ActivationFunctionType.Ln
    Copy = mybir.ActivationFunctionType.Copy

    nc = bacc.Bacc("TRN2", target_bir_lowering=False, debug=False,
                   num_devices=N_CORES)

    xt_d = nc.dram_tensor("xt", [C, T], bf16, kind="ExternalInput").ap()
    wq_d = nc.dram_tensor("wq", [C, 256], bf16, kind="ExternalInput").ap()
    wk_d = nc.dram_tensor("wk", [C, 256], bf16, kind="ExternalInput").ap()
    wv_d = nc.dram_tensor("wv", [C, 256], bf16, kind="ExternalInput").ap()
    wp_d = nc.dram_tensor("wp", [256, C], bf16, kind="ExternalInput").ap()
    cosr_d = nc.dram_tensor("cosr", [P, T], bf16, kind="ExternalInput").ap()
    sinr_d = nc.dram_tensor("sinr", [P, T], bf16, kind="ExternalInput").ap()
    ind32_d = nc.dram_tensor("ind32", [P, HPC], bf16, kind="ExternalInput").ap()
    bc32_d = nc.dram_tensor("bc32", [HPC, P], bf16, kind="ExternalInput").ap()
    selpair4_d = nc.dram_tensor("selpair4", [HPC, 2 * P], bf16, kind="ExternalInput").ap()
    maskt_d = nc.dram_tensor("maskt", [P, 2, P], bf16, kind="ExternalInput").ap()
    out_d = nc.dram_tensor("outT", [C, T], bf16, kind="ExternalOutput").ap()

    with tile.TileContext(nc) as tc:
        with (
            tc.tile_pool(name="sb", bufs=1) as sb,
            tc.tile_pool(name="trans", bufs=2) as tr,
            tc.tile_pool(name="ps", bufs=1, space="PSUM") as ps,
        ):
            def direct_load(name, dram_slice, shape, dt):
                t_r = sb.tile(shape, dt, tag=name, name=name)
                nc.sync.dma_start(out=t_r[:], in_=dram_slice)
                return t_r

            # ---- tiny consts first ----
            ind32_r = direct_load("ind32r", ind32_d[:, :], [P, HPC], bf16)
            bc32_r = direct_load("bc32r", bc32_d[:, :], [HPC, P], bf16)
            selpair_r = direct_load("selpairr", selpair4_d[:, :], [HPC, 2 * P], bf16)
            mask_r = direct_load("maskr", maskt_d[:, :, :], [P, 2, P], bf16)

            eps_t = sb.tile([HPC, 1], f32, tag="epst", name="eps_t")
            nc.gpsimd.memset(eps_t[:], RMS_EPS)

            # q/k weight tiles; DMAs issued interleaved with block-0 x below
            wq_r = [sb.tile([P, 256], bf16, tag=f"wqr{k}", name=f"wqr{k}")
                    for k in range(KCH)]
            wk_r = [sb.tile([P, 256], bf16, tag=f"wkr{k}", name=f"wkr{k}")
                    for k in range(KCH)]

            # ---- persistent intermediates ----
            q1 = [sb.tile([P, TB], bf16, tag=f"q1_{n}", name=f"q1_{n}")
                  for n in range(NB)]
            q2 = [sb.tile([P, TB], bf16, tag=f"q2_{n}", name=f"q2_{n}")
                  for n in range(NB)]
            k1 = [sb.tile([P, TB], bf16, tag=f"k1_{n}", name=f"k1_{n}")
                  for n in range(NB)]
            k2 = [sb.tile([P, TB], bf16, tag=f"k2_{n}", name=f"k2_{n}")
                  for n in range(NB)]
            v_r = [sb.tile([P, HPC, 65], bf16, tag=f"v{s}", name=f"v{s}")
                   for s in range(T // P)]
            for s in range(T // P):
                nc.gpsimd.memset(v_r[s][:, :, 64:65], 1.0)
            yT32 = [sb.tile([P, T], f32, tag=f"yT32_{c}", name=f"yT32_{c}")
                    for c in range(2)]
            yT_bf = [sb.tile([P, T], bf16, tag=f"yTbf_{c}", name=f"yTbf_{c}")
                     for c in range(2)]
            den4 = sb.tile([HPC, T], f32, tag="den4", name="den4")
            invden4 = sb.tile([HPC, T], f32, tag="invden4", name="invden4")
            invden_bf = sb.tile([HPC, T], bf16, tag="invdenbf", name="invden_bf")
            recscr = sb.tile([HPC, T], f32, tag="recscr", name="recscr")

            # PSUM tags: SS [128,2,512] x2 bufs (4 banks), YA/YB [65,512]
            # (2 banks), PF [128,512] x2 bufs (2 banks) = 8 banks exactly.

            # ====== Phase 1: projections; RMS+rope deferred one block ======
            deferred = None
            for n in range(NB):
                nsl = slice(n * TB, (n + 1) * TB)
                xr_t = []
                for k in range(KCH):
                    xr = tr.tile([P, TB], bf16, tag="xr", name=f"xr{n}_{k}", bufs=16)
                    if n == 0:
                        nc.sync.dma_start(out=wq_r[k][:],
                                          in_=wq_d[k * P:(k + 1) * P, :])
                        nc.sync.dma_start(out=wk_r[k][:],
                                          in_=wk_d[k * P:(k + 1) * P, :])
                    nc.sync.dma_start(out=xr[:], in_=xt_d[k * P:(k + 1) * P, nsl])
                    xr_t.append(xr)
                if n == 0:
                    # not needed until later: load behind the block-0 stream
                    wv_r = [direct_load(f"wvr{k}", wv_d[k * P:(k + 1) * P, :],
                                        [P, 256], bf16) for k in range(KCH)]
                    cosr_t = direct_load("cosr", cosr_d[:, :], [P, T], bf16)
                    sinr_t = direct_load("sinr", sinr_d[:, :], [P, T], bf16)
                    wp_r = [direct_load(f"wpr{c}", wp_d[c * P:(c + 1) * P, :],
                                        [P, C], bf16) for c in range(2)]
                pq = ps.tile([P, 2, TB], f32, tag="SS", name=f"pq_{n}", bufs=2)
                pk = ps.tile([P, 2, TB], f32, tag="SS", name=f"pk_{n}", bufs=2)
                for k in range(KCH):
                    xr = xr_t[k]
                    st = (k == 0)
                    sp = (k == KCH - 1)
                    nc.tensor.matmul(pq[:, 0, :], lhsT=wq_r[k][:, 0:128], rhs=xr[:],
                                     start=st, stop=sp)
                    nc.tensor.matmul(pq[:, 1, :], lhsT=wq_r[k][:, 128:256], rhs=xr[:],
                                     start=st, stop=sp)
                    nc.tensor.matmul(pk[:, 0, :], lhsT=wk_r[k][:, 0:128], rhs=xr[:],
                                     start=st, stop=sp)
                    nc.tensor.matmul(pk[:, 1, :], lhsT=wk_r[k][:, 128:256], rhs=xr[:],
                                     start=st, stop=sp)
                # fused PSUM -> SBUF copies, bf16 out (2x DVE rate downstream)
                x12q = tr.tile([P, 2, TB], bf16, tag="x12q", name=f"x12q{n}", bufs=1)
                x12k = tr.tile([P, 2, TB], bf16, tag="x12k", name=f"x12k{n}", bufs=1)
                nc.vector.tensor_copy(x12q[:], pq[:])
                nc.vector.tensor_copy(x12k[:], pk[:])
                # v projections (token-major: lhsT = x chunk)
                pv = [ps.tile([P, 256], f32, tag=("YA", "YB")[s % 2],
                              name=f"pv{n}_{s}") for s in range(4)]
                for k in range(KCH):
                    st = (k == 0)
                    sp = (k == KCH - 1)
                    for s_rel in range(4):
                        nc.tensor.matmul(
                            pv[s_rel][:],
                            lhsT=xr_t[k][:, s_rel * P:(s_rel + 1) * P],
                            rhs=wv_r[k][:], start=st, stop=sp)
                for s_rel in range(4):
                    nc.vector.tensor_copy(v_r[4 * n + s_rel][:, :, 0:64],
                                          pv[s_rel][:])
                # RMS sums + Ln/Exp (scalar); rope runs NOW on raw x (it is
                # linear, so the 1/rms scale is applied after, deferred one
                # block so the scalar Ln/Exp latency never stalls the PE)
                invcs = {}
                lns = {}
                for (x12, eng) in ((x12q, "q"), (x12k, "k")):
                    e = nc.vector if eng == "q" else nc.gpsimd
                    ta = "sq" + eng
                    sq12 = tr.tile([P, 2, TB], bf16, tag=ta, name=f"sq{eng}{n}", bufs=1)
                    e.tensor_mul(sq12[:], x12[:], x12[:])
                    ps_s = ps.tile([HPC, TB], f32, tag="PF",
                                   name=f"pss{eng}{n}", bufs=2)
                    nc.tensor.matmul(ps_s[:], lhsT=ind32_r[:], rhs=sq12[:, 0, :],
                                     start=True, stop=False)
                    nc.tensor.matmul(ps_s[:], lhsT=ind32_r[:], rhs=sq12[:, 1, :],
                                     start=False, stop=True)
                    lnm = tr.tile([HPC, TB], f32, tag="lnm" + eng,
                                  name=f"lnm{eng}{n}", bufs=1)
                    lns[eng] = (ps_s, lnm)
                for eng in ("q", "k"):
                    ps_s, lnm = lns[eng]
                    nc.scalar.activation(lnm[:], ps_s[:], Ln,
                                         bias=eps_t[:], scale=1.0 / 64.0)
                for eng in ("q", "k"):
                    ps_s, lnm = lns[eng]
                    invc = tr.tile([HPC, TB], bf16, tag="invc" + eng,
                                   name=f"invc{eng}{n}")
                    nc.scalar.activation(invc[:], lnm[:], Exp, scale=-0.5)
                    invcs[eng] = invc
                # rope immediately (raw, un-normalized), bf16 2x DVE rate
                for (x12, d1, d2, eng) in ((x12q, q1, q2, "q"),
                                           (x12k, k1, k2, "k")):
                    e = nc.vector if eng == "q" else nc.gpsimd
                    ta, tb = (("rtA", "rtB") if eng == "q" else ("rtC", "rtD"))
                    m_a = tr.tile([P, TB], bf16, tag=ta, name=f"ma{eng}{n}", bufs=1)
                    m_b = tr.tile([P, TB], bf16, tag=tb, name=f"mb{eng}{n}", bufs=1)
                    e.tensor_mul(m_a[:], x12[:, 0, :], cosr_t[:, nsl])
                    e.tensor_mul(m_b[:], x12[:, 1, :], sinr_t[:, nsl])
                    e.tensor_add(d1[n][:], m_a[:], m_b[:])
                    m_c = tr.tile([P, TB], bf16, tag=ta, name=f"mc{eng}{n}", bufs=1)
                    m_d = tr.tile([P, TB], bf16, tag=tb, name=f"md{eng}{n}", bufs=1)
                    e.tensor_mul(m_c[:], x12[:, 1, :], cosr_t[:, nsl])
                    e.tensor_mul(m_d[:], x12[:, 0, :], sinr_t[:, nsl])
                    e.tensor_sub(d2[n][:], m_c[:], m_d[:])

                def make_deferred(n=n, invcs=invcs):
                    def run():
                        for (d1, d2, eng) in ((q1, q2, "q"), (k1, k2, "k")):
                            e = nc.vector if eng == "q" else nc.gpsimd
                            ps_b = ps.tile([P, TB], f32, tag="PF",
                                           name=f"psb{eng}{n}", bufs=2)
                            nc.tensor.matmul(ps_b[:], lhsT=bc32_r[:],
                                             rhs=invcs[eng][:],
                                             start=True, stop=True)
                            pb_bf = tr.tile([P, TB], bf16, tag="pbbf" + eng,
                                            name=f"pbbf{eng}{n}", bufs=1)
                            nc.vector.tensor_copy(pb_bf[:], ps_b[:])
                            e.tensor_mul(d1[n][:], d1[n][:], pb_bf[:])
                            e.tensor_mul(d2[n][:], d2[n][:], pb_bf[:])
                    return run

                if deferred is not None:
                    deferred()
                deferred = make_deferred()
            deferred()

            # ====== Phase 2: attention; normalize+out-proj of block j
            # deferred into block j-1's attention stream as per-chunk
            # fillers (keeps the PE stream gapless so HAM stays at full
            # clock), with any remainder as a dense burst at j end ======
            def norm_filler(j, c):
                jsl = slice(j * TB, (j + 1) * TB)

                def run():
                    ps_i = ps.tile([P, TB], f32, tag="PF",
                                   name=f"psi{c}{j}", bufs=2)
                    nc.tensor.matmul(ps_i[:],
                                     lhsT=selpair_r[:, c * P:(c + 1) * P],
                                     rhs=invden_bf[:, jsl],
                                     start=True, stop=True)
                    nc.vector.tensor_mul(yT_bf[c][:, jsl],
                                         yT32[c][:, jsl], ps_i[:])
                return run

            def po_filler(j, o):
                jsl = slice(j * TB, (j + 1) * TB)

                def run():
                    osl = slice(o * P, (o + 1) * P)
                    po = ps.tile([P, TB], f32, tag="PF",
                                 name=f"po{o}_{j}", bufs=2)
                    nc.tensor.matmul(po[:], lhsT=wp_r[0][:, osl],
                                     rhs=yT_bf[0][:, jsl],
                                     start=True, stop=False)
                    nc.tensor.matmul(po[:], lhsT=wp_r[1][:, osl],
                                     rhs=yT_bf[1][:, jsl],
                                     start=False, stop=True)
                    ob = tr.tile([P, TB], bf16, tag="ob",
                                 name=f"ob{o}_{j}", bufs=3)
                    nc.vector.tensor_copy(ob[:], po[:])
                    nc.sync.dma_start(out=out_d[osl, jsl], in_=ob[:])
                return run

            def make_fillers(j):
                return ([norm_filler(j, c) for c in range(2)]
                        + [po_filler(j, o) for o in range(8)])

            # j descending: the dense j=3 attention stream right after P1
            # keeps the PE warm; small-j blocks at the end coincide with the
            # out-projection fillers
            fillers = []
            for j in reversed(range(NB)):
                jsl = slice(j * TB, (j + 1) * TB)
                n_k = 4 * j + 4
                for c in range(2):
                    Yh = [ps.tile([65, TB], f32, tag=("YA", "YB")[a],
                                  name=f"Y{c}_{a}_{j}") for a in range(2)]
                    S_tiles = [None] * n_k

                    def issue_S(k, c=c, j=j, S_tiles=S_tiles):
                        r = k - 4 * j
                        mtrim = 128 * r if r > 0 else 0
                        msl = slice(mtrim, TB)
                        nb_, kc = k // 4, k % 4
                        ksl = slice(128 * kc, 128 * kc + 128)
                        S_t = ps.tile([P, 2, TB], f32, tag="SS",
                                      name=f"S{c}_{j}_{k}", bufs=2)
                        for a in range(2):
                            h = 2 * c + a
                            hsl = slice(32 * h, 32 * h + 32)
                            nc.tensor.matmul(S_t[:, a, msl],
                                             lhsT=k1[nb_][hsl, ksl],
                                             rhs=q1[j][hsl, msl],
                                             start=True, stop=False,
                                             tile_position=(32 * h, 0))
                            nc.tensor.matmul(S_t[:, a, msl],
                                             lhsT=k2[nb_][hsl, ksl],
                                             rhs=q2[j][hsl, msl],
                                             start=False, stop=True,
                                             tile_position=(32 * h, 0))
                        S_tiles[k] = S_t

                    issue_S(0)
                    for k in range(n_k):
                        r = k - 4 * j
                        mtrim = 128 * r if r > 0 else 0
                        msl = slice(mtrim, TB)
                        S_t = S_tiles[k]
                        e_t = tr.tile([P, 2, TB], bf16, tag="eS",
                                      name=f"e{c}_{j}_{k}", bufs=3)
                        nc.scalar.activation(e_t[:, :, msl], S_t[:, :, msl],
                                             Exp, scale=0.125)
                        if r >= 0:
                            tsl = slice(128 * r, 128 * r + 128)
                            nc.gpsimd.tensor_mul(e_t[:, :, tsl], e_t[:, :, tsl],
                                                 mask_r[:])
                        if k + 1 < n_k:
                            issue_S(k + 1)
                        if fillers:
                            fillers.pop(0)()
                        st, sp = (k == 0), (k == n_k - 1)
                        for a in range(2):
                            nc.tensor.matmul(Yh[a][:, msl],
                                             lhsT=v_r[k][:, 2 * c + a, :],
                                             rhs=e_t[:, a, msl],
                                             start=st, stop=sp)
                    for a in range(2):
                        h = 2 * c + a
                        yb = tr.tile([65, TB], f32, tag="cpbuf",
                                     name=f"yb{h}_{j}", bufs=4,
                                     padded_shape=[P, TB])
                        nc.vector.tensor_copy(yb[:], Yh[a][:])
                        nc.sync.dma_start(out=yT32[c][64 * a:64 * a + 64, jsl],
                                          in_=yb[0:64, :])
                        nc.sync.dma_start(out=den4[h:h + 1, jsl],
                                          in_=yb[64:65, :])
                # 1/den (fast approx is plenty for a softmax denominator);
                # bf16 round via bitcast view trick is not possible for
                # matmul rhs, so store bf16 copy inline
                nc.vector.reciprocal_approx_accurate(
                    out=invden4[:, jsl], in_=den4[:, jsl],
                    scratch=recscr[:, jsl])
                nc.vector.tensor_copy(invden_bf[:, jsl], invden4[:, jsl])
                while fillers:          # drain leftovers as a dense burst
                    fillers.pop(0)()
                fillers = make_fillers(j)
            while fillers:
                fillers.pop(0)()

    nc.compile()
    return nc


def _get_module():
    if "nc" not in _CACHE:
        _CACHE["nc"] = _build_module()
        _CACHE["consts"] = _build_consts()
    return _CACHE["nc"], _CACHE["consts"]


def _bf16(a):
    return np.ascontiguousarray(a, dtype=np.float32).astype(ml_dtypes.bfloat16)


def _core_inputs(x, w_q, w_k, w_v, w_proj, core):
    """Build the per-core input map (numpy, host-side sharding)."""
    b = core // 4
    g = core % 4
    heads = [4 * g + j for j in range(HPC)]

    xt = _bf16(np.ascontiguousarray(x[b].T))              # [C, T]

    perm = np.empty(256, dtype=np.int64)
    for m in range(128):
        perm[m] = 64 * heads[m // 32] + (m % 32)             # x1 half
        perm[128 + m] = 64 * heads[m // 32] + 32 + (m % 32)  # x2 half
    wq = _bf16(np.ascontiguousarray(w_q[perm, :].T))         # [C, 256]
    wk = _bf16(np.ascontiguousarray(w_k[perm, :].T))

    vperm = np.empty(256, dtype=np.int64)
    for m in range(256):
        vperm[m] = 64 * heads[m // 64] + (m % 64)
    wv = _bf16(np.ascontiguousarray(w_v[vperm, :].T))        # [C, 256]
    wp = _bf16(np.ascontiguousarray(w_proj[:, vperm].T))     # [256, C]

    return dict(xt=xt, wq=wq, wk=wk, wv=wv, wp=wp)


def kernel(x, w_q, w_k, w_v, w_proj, _trace=False, _trace_cores=None):
    from concourse.bass_utils import run_bass_kernel_spmd

    nc, consts = _get_module()
    x = np.asarray(x, dtype=np.float32)
    in_maps = []
    for core in range(N_CORES):
        m = _core_inputs(np.asarray(x), np.asarray(w_q), np.asarray(w_k),
                         np.asarray(w_v), np.asarray(w_proj), core)
        m.update(consts)
        in_maps.append(m)

    res = run_bass_kernel_spmd(nc, in_maps, list(range(N_CORES)),
                               trace=_trace, trace_cores=_trace_cores)
    outs = [np.asarray(res.results[c]["outT"], dtype=np.float32)
            for c in range(N_CORES)]
    out = np.empty((B, T, C), dtype=np.float32)
    for b in range(B):
        acc = outs[4 * b]
        for g in range(1, 4):
            acc = acc + outs[4 * b + g]
        out[b] = acc.T
    if _trace:
        kernel._last_exec_time_ns = res.exec_time_ns
        kernel._last_results = res
    return out


# revision 62
# speedup vs baseline: 1.0327x; 1.0327x over previous
"""Causal self-attention (RMSNorm-QK + RoPE) Trainium2 Bass kernel.

Problem: B=2, T=2048, C=1024, H=16 heads, D=64.
Sharding: 8 cores = 2 (batch) x 4 (head groups of 4 heads).
Each core computes q/k/v projections for its 4 heads, attention, and a
partial output projection (column-parallel over heads); the host sums the
4 partials per batch and transposes.

All matmuls run in bf16 (inputs rounded on host) with f32 PSUM accumulation.
bf16 halves HBM traffic vs f32r, draws less PE power (less HAM throttle),
and runs full-rate at any free size. Avoid f32r DMA loads entirely: the
f32r-rounding DMA pass truncates mantissas over a wider SBUF region than
its own tile and corrupts bf16 neighbours.

Per-core layouts:
  projection chunks [128, 512]: row 32h+i = head h, rope-half dim i
  q1/q2/k1/k2[n]  [128, 512] bf16 : rope outputs per t-block, kept in the
      32h+i row layout; scores contract rc1+rc2 with two K=32 matmuls per
      head at PE row-group 32h (heads of a pair run concurrently).
  v_r[s]          [128, 4, 65] bf16 : key-chunk s, head h at [:, h, 0:64],
      ones column at [:, h, 64] (softmax denominator trick)
  S (tag SS)      [128, 2, 512] f32 PSUM : scores for one head pair
  yT32[c]         [128, 2048] f32 : heads (2c, 2c+1) attention numerator
  yT_bf[c]        [128, 2048] bf16 : normalized (divided by denominator)
Output: outT [1024, 2048] bf16 = (partial out).T per core; host sums.

Pipelining: block n's RMS+rope tensor/vector work is deferred until block
n+1's projection matmuls are issued (the scalar Ln/Exp latency hides under
them); block j's normalize+out-projection matmuls are spread as fillers
into block j+1's attention stream so the tensor engine never idles long
enough to trip the HAM half-clock throttle.
"""

import sys

for _p in ("/opt/trn_rl_repo",):
    if _p not in sys.path:
        sys.path.append(_p)

import numpy as np
import ml_dtypes

B, T, C = 2, 2048, 1024
H_TOT, D = 16, 64
HPC = 4               # heads per core
N_CORES = 8
P = 128               # partitions
NB = 4                # t-blocks of 512
TB = 512              # t-block size
KCH = 8               # C / 128 contraction chunks
RMS_EPS = 1.1920928955078125e-07
ROPE_BASE = 10000.0

_CACHE = {}


def _build_consts():
    """Host-side constant tensors shared by all cores."""
    inv_freq = (1.0 / (ROPE_BASE ** (np.arange(0, D, 2, dtype=np.float32) / np.float32(D)))).astype(np.float32)
    pos = np.arange(T, dtype=np.float32)
    freqs = np.outer(pos, inv_freq).astype(np.float32)      # [T, 32]
    cos = np.cos(freqs).astype(np.float32)                  # [T, 32]
    sin = np.sin(freqs).astype(np.float32)
    cosr = np.ascontiguousarray(np.tile(cos.T, (HPC, 1))).astype(ml_dtypes.bfloat16)
    sinr = np.ascontiguousarray(np.tile(sin.T, (HPC, 1))).astype(ml_dtypes.bfloat16)
    # ind32 [128, 4]: per-32-row-group summing matrix (lhsT for RMS sums)
    ind32 = np.zeros((P, HPC), dtype=np.float32)
    for p_ in range(P):
        ind32[p_, p_ // 32] = 1.0
    # bc32 [4, 128]: broadcast inv (4 heads) to 32-row groups (lhsT)
    bc32 = np.zeros((HPC, P), dtype=np.float32)
    for p_ in range(P):
        bc32[p_ // 32, p_] = 1.0
    # selpair4 [4, 256]: pair c: out row m <- den row (2c + m//64)
    selpair4 = np.zeros((HPC, 2 * P), dtype=np.float32)
    for c in range(2):
        for m in range(P):
            selpair4[2 * c + m // 64, 128 * c + m] = 1.0
    # causal triangle mask [128, 2, 128] bf16 (same triangle both halves):
    # keep element (p, :, i) iff i >= p
    tri = (np.arange(P)[None, :] >= np.arange(P)[:, None]).astype(np.float32)
    maskt = np.ascontiguousarray(
        np.broadcast_to(tri[:, None, :], (P, 2, P))).astype(ml_dtypes.bfloat16)
    bf = ml_dtypes.bfloat16
    return dict(cosr=cosr, sinr=sinr, ind32=ind32.astype(bf),
                bc32=bc32.astype(bf), selpair4=selpair4.astype(bf),
                maskt=maskt)


def _build_module():
    import concourse.bacc as bacc
    import concourse.mybir as mybir
    import concourse.tile as tile

    f32 = mybir.dt.float32
    bf16 = mybir.dt.bfloat16
    Exp = mybir.ActivationFunctionType.Exp
    Ln = mybir.ActivationFunctionType.Ln
    Copy = mybir.ActivationFunctionType.Copy

    nc = bacc.Bacc("TRN2", target_bir_lowering=False, debug=False,
                   num_devices=N_CORES)

    xt_d = nc.dram_tensor("xt", [C, T], bf16, kind="ExternalInput").ap()
    wq_d = nc.dram_tensor("wq", [C, 256], bf16, kind="ExternalInput").ap()
    wk_d = nc.dram_tensor("wk", [C, 256], bf16, kind="ExternalInput").ap()
    wv_d = nc.dram_tensor("wv", [C, 256], bf16, kind="ExternalInput").ap()
    wp_d = nc.dram_tensor("wp", [256, C], bf16, kind="ExternalInput").ap()
    cosr_d = nc.dram_tensor("cosr", [P, T], bf16, kind="ExternalInput").ap()
    sinr_d = nc.dram_tensor("sinr", [P, T], bf16, kind="ExternalInput").ap()
    ind32_d = nc.dram_tensor("ind32", [P, HPC], bf16, kind="ExternalInput").ap()
    bc32_d = nc.dram_tensor("bc32", [HPC, P], bf16, kind="ExternalInput").ap()
    selpair4_d = nc.dram_tensor("selpair4", [HPC, 2 * P], bf16, kind="ExternalInput").ap()
    maskt_d = nc.dram_tensor("maskt", [P, 2, P], bf16, kind="ExternalInput").ap()
    out_d = nc.dram_tensor("outT", [C, T], bf16, kind="ExternalOutput").ap()

    with tile.TileContext(nc) as tc:
        with (
            tc.tile_pool(name="sb", bufs=1) as sb,
            tc.tile_pool(name="trans", bufs=2) as tr,
            tc.tile_pool(name="ps", bufs=1, space="PSUM") as ps,
        ):
            def direct_load(name, dram_slice, shape, dt):
                t_r = sb.tile(shape, dt, tag=name, name=name)
                nc.sync.dma_start(out=t_r[:], in_=dram_slice)
                return t_r

            # ---- tiny consts first ----
            ind32_r = direct_load("ind32r", ind32_d[:, :], [P, HPC], bf16)
            bc32_r = direct_load("bc32r", bc32_d[:, :], [HPC, P], bf16)
            selpair_r = direct_load("selpairr", selpair4_d[:, :], [HPC, 2 * P], bf16)
            mask_r = direct_load("maskr", maskt_d[:, :, :], [P, 2, P], bf16)

            eps_t = sb.tile([HPC, 1], f32, tag="epst", name="eps_t")
            nc.gpsimd.memset(eps_t[:], RMS_EPS)

            # q/k weight tiles; DMAs issued interleaved with block-0 x below
            wq_r = [sb.tile([P, 256], bf16, tag=f"wqr{k}", name=f"wqr{k}")
                    for k in range(KCH)]
            wk_r = [sb.tile([P, 256], bf16, tag=f"wkr{k}", name=f"wkr{k}")
                    for k in range(KCH)]

            # ---- persistent intermediates ----
            q1 = [sb.tile([P, TB], bf16, tag=f"q1_{n}", name=f"q1_{n}")
                  for n in range(NB)]
            q2 = [sb.tile([P, TB], bf16, tag=f"q2_{n}", name=f"q2_{n}")
                  for n in range(NB)]
            k1 = [sb.tile([P, TB], bf16, tag=f"k1_{n}", name=f"k1_{n}")
                  for n in range(NB)]
            k2 = [sb.tile([P, TB], bf16, tag=f"k2_{n}", name=f"k2_{n}")
                  for n in range(NB)]
            v_r = [sb.tile([P, HPC, 65], bf16, tag=f"v{s}", name=f"v{s}")
                   for s in range(T // P)]
            for s in range(T // P):
                nc.gpsimd.memset(v_r[s][:, :, 64:65], 1.0)
            yT32 = [sb.tile([P, T], f32, tag=f"yT32_{c}", name=f"yT32_{c}")
                    for c in range(2)]
            yT_bf = [sb.tile([P, T], bf16, tag=f"yTbf_{c}", name=f"yTbf_{c}")
                     for c in range(2)]
            den4 = sb.tile([HPC, T], f32, tag="den4", name="den4")
            invden4 = sb.tile([HPC, T], f32, tag="invden4", name="invden4")
            invden_bf = sb.tile([HPC, T], bf16, tag="invdenbf", name="invden_bf")
            recscr = sb.tile([HPC, T], f32, tag="recscr", name="recscr")

            # PSUM tags: SS [128,2,512] x2 bufs (4 banks), YA/YB [65,512]
            # (2 banks), PF [128,512] x2 bufs (2 banks) = 8 banks exactly.

            # ====== Phase 1: projections; RMS+rope deferred one block ======
            deferred = None
            for n in range(NB):
                nsl = slice(n * TB, (n + 1) * TB)
                xr_t = []
                for k in range(KCH):
                    xr = tr.tile([P, TB], bf16, tag="xr", name=f"xr{n}_{k}", bufs=16)
                    if n == 0:
                        nc.sync.dma_start(out=wq_r[k][:],
                                          in_=wq_d[k * P:(k + 1) * P, :])
                        nc.sync.dma_start(out=wk_r[k][:],
                                          in_=wk_d[k * P:(k + 1) * P, :])
                    nc.sync.dma_start(out=xr[:], in_=xt_d[k * P:(k + 1) * P, nsl])
                    xr_t.append(xr)
                if n == 0:
                    # not needed until later: load behind the block-0 stream
                    wv_r = [direct_load(f"wvr{k}", wv_d[k * P:(k + 1) * P, :],
                                        [P, 256], bf16) for k in range(KCH)]
                    cosr_t = direct_load("cosr", cosr_d[:, :], [P, T], bf16)
                    sinr_t = direct_load("sinr", sinr_d[:, :], [P, T], bf16)
                    wp_r = [direct_load(f"wpr{c}", wp_d[c * P:(c + 1) * P, :],
                                        [P, C], bf16) for c in range(2)]
                pq = ps.tile([P, 2, TB], f32, tag="SS", name=f"pq_{n}", bufs=2)
                pk = ps.tile([P, 2, TB], f32, tag="SS", name=f"pk_{n}", bufs=2)
                for k in range(KCH):
                    xr = xr_t[k]
                    st = (k == 0)
                    sp = (k == KCH - 1)
                    nc.tensor.matmul(pq[:, 0, :], lhsT=wq_r[k][:, 0:128], rhs=xr[:],
                                     start=st, stop=sp)
                    nc.tensor.matmul(pq[:, 1, :], lhsT=wq_r[k][:, 128:256], rhs=xr[:],
                                     start=st, stop=sp)
                    nc.tensor.matmul(pk[:, 0, :], lhsT=wk_r[k][:, 0:128], rhs=xr[:],
                                     start=st, stop=sp)
                    nc.tensor.matmul(pk[:, 1, :], lhsT=wk_r[k][:, 128:256], rhs=xr[:],
                                     start=st, stop=sp)
                # fused PSUM -> SBUF copies, bf16 out (2x DVE rate downstream)
                x12q = tr.tile([P, 2, TB], bf16, tag="x12q", name=f"x12q{n}", bufs=1)
                x12k = tr.tile([P, 2, TB], bf16, tag="x12k", name=f"x12k{n}", bufs=1)
                nc.vector.tensor_copy(x12q[:], pq[:])
                nc.vector.tensor_copy(x12k[:], pk[:])
                # v projections (token-major: lhsT = x chunk)
                pv = [ps.tile([P, 256], f32, tag=("YA", "YB")[s % 2],
                              name=f"pv{n}_{s}") for s in range(4)]
                for k in range(KCH):
                    st = (k == 0)
                    sp = (k == KCH - 1)
                    for s_rel in range(4):
                        nc.tensor.matmul(
                            pv[s_rel][:],
                            lhsT=xr_t[k][:, s_rel * P:(s_rel + 1) * P],
                            rhs=wv_r[k][:], start=st, stop=sp)
                for s_rel in range(4):
                    nc.vector.tensor_copy(v_r[4 * n + s_rel][:, :, 0:64],
                                          pv[s_rel][:])
                # RMS sums + Ln/Exp (scalar); rope runs NOW on raw x (it is
                # linear, so the 1/rms scale is applied after, deferred one
                # block so the scalar Ln/Exp latency never stalls the PE)
                invcs = {}
                lns = {}
                for (x12, eng) in ((x12q, "q"), (x12k, "k")):
                    e = nc.vector if eng == "q" else nc.gpsimd
                    ta = "sq" + eng
                    sq12 = tr.tile([P, 2, TB], bf16, tag=ta, name=f"sq{eng}{n}", bufs=1)
                    e.tensor_mul(sq12[:], x12[:], x12[:])
                    ps_s = ps.tile([HPC, TB], f32, tag="PF",
                                   name=f"pss{eng}{n}", bufs=2)
                    nc.tensor.matmul(ps_s[:], lhsT=ind32_r[:], rhs=sq12[:, 0, :],
                                     start=True, stop=False)
                    nc.tensor.matmul(ps_s[:], lhsT=ind32_r[:], rhs=sq12[:, 1, :],
                                     start=False, stop=True)
                    lnm = tr.tile([HPC, TB], f32, tag="lnm" + eng,
                                  name=f"lnm{eng}{n}", bufs=1)
                    lns[eng] = (ps_s, lnm)
                for eng in ("q", "k"):
                    ps_s, lnm = lns[eng]
                    nc.scalar.activation(lnm[:], ps_s[:], Ln,
                                         bias=eps_t[:], scale=1.0 / 64.0)
                for eng in ("q", "k"):
                    ps_s, lnm = lns[eng]
                    invc = tr.tile([HPC, TB], bf16, tag="invc" + eng,
                                   name=f"invc{eng}{n}")
                    nc.scalar.activation(invc[:], lnm[:], Exp, scale=-0.5)
                    invcs[eng] = invc
                # rope immediately (raw, un-normalized), bf16 2x DVE rate
                for (x12, d1, d2, eng) in ((x12q, q1, q2, "q"),
                                           (x12k, k1, k2, "k")):
                    e = nc.vector if eng == "q" else nc.gpsimd
                    ta, tb = (("rtA", "rtB") if eng == "q" else ("rtC", "rtD"))
                    m_a = tr.tile([P, TB], bf16, tag=ta, name=f"ma{eng}{n}", bufs=1)
                    m_b = tr.tile([P, TB], bf16, tag=tb, name=f"mb{eng}{n}", bufs=1)
                    e.tensor_mul(m_a[:], x12[:, 0, :], cosr_t[:, nsl])
                    e.tensor_mul(m_b[:], x12[:, 1, :], sinr_t[:, nsl])
                    e.tensor_add(d1[n][:], m_a[:], m_b[:])
                    m_c = tr.tile([P, TB], bf16, tag=ta, name=f"mc{eng}{n}", bufs=1)
                    m_d = tr.tile([P, TB], bf16, tag=tb, name=f"md{eng}{n}", bufs=1)
                    e.tensor_mul(m_c[:], x12[:, 1, :], cosr_t[:, nsl])
                    e.tensor_mul(m_d[:], x12[:, 0, :], sinr_t[:, nsl])
                    e.tensor_sub(d2[n][:], m_c[:], m_d[:])

                def make_deferred(n=n, invcs=invcs):
                    def run():
                        for (d1, d2, eng) in ((q1, q2, "q"), (k1, k2, "k")):
                            e = nc.vector if eng == "q" else nc.gpsimd
                            ps_b = ps.tile([P, TB], f32, tag="PF",
                                           name=f"psb{eng}{n}", bufs=2)
                            nc.tensor.matmul(ps_b[:], lhsT=bc32_r[:],
                                             rhs=invcs[eng][:],
                                             start=True, stop=True)
                            pb_bf = tr.tile([P, TB], bf16, tag="pbbf" + eng,
                                            name=f"pbbf{eng}{n}", bufs=1)
                            nc.vector.tensor_copy(pb_bf[:], ps_b[:])
                            e.tensor_mul(d1[n][:], d1[n][:], pb_bf[:])
                            e.tensor_mul(d2[n][:], d2[n][:], pb_bf[:])
                    return run

                if deferred is not None:
                    deferred()
                deferred = make_deferred()
            deferred()

            # ====== Phase 2: attention; normalize+out-proj of block j
            # deferred into block j-1's attention stream as per-chunk
            # fillers (keeps the PE stream gapless so HAM stays at full
            # clock), with any remainder as a dense burst at j end ======
            def norm_filler(j, c):
                jsl = slice(j * TB, (j + 1) * TB)

                def run():
                    ps_i = ps.tile([P, TB], f32, tag="PF",
                                   name=f"psi{c}{j}", bufs=2)
                    nc.tensor.matmul(ps_i[:],
                                     lhsT=selpair_r[:, c * P:(c + 1) * P],
                                     rhs=invden_bf[:, jsl],
                                     start=True, stop=True)
                    nc.vector.tensor_mul(yT_bf[c][:, jsl],
                                         yT32[c][:, jsl], ps_i[:])
                return run

            def po_filler(j, o):
                jsl = slice(j * TB, (j + 1) * TB)

                def run():
                    osl = slice(o * P, (o + 1) * P)
                    po = ps.tile([P, TB], f32, tag="PF",
                                 name=f"po{o}_{j}", bufs=2)
                    nc.tensor.matmul(po[:], lhsT=wp_r[0][:, osl],
                                     rhs=yT_bf[0][:, jsl],
                                     start=True, stop=False)
                    nc.tensor.matmul(po[:], lhsT=wp_r[1][:, osl],
                                     rhs=yT_bf[1][:, jsl],
                                     start=False, stop=True)
                    ob = tr.tile([P, TB], bf16, tag="ob",
                                 name=f"ob{o}_{j}", bufs=3)
                    nc.vector.tensor_copy(ob[:], po[:])
                    nc.sync.dma_start(out=out_d[osl, jsl], in_=ob[:])
                return run

            def make_fillers(j):
                return ([norm_filler(j, c) for c in range(2)]
                        + [po_filler(j, o) for o in range(8)])

            # j descending: the dense j=3 attention stream right after P1
            # keeps the PE warm; small-j blocks at the end coincide with the
            # out-projection fillers
            fillers = []
            for j in reversed(range(NB)):
                jsl = slice(j * TB, (j + 1) * TB)
                n_k = 4 * j + 4
                for c in range(2):
                    Yh = [ps.tile([65, TB], f32, tag=("YA", "YB")[a],
                                  name=f"Y{c}_{a}_{j}") for a in range(2)]
                    S_tiles = [None] * n_k

                    def issue_S(k, c=c, j=j, S_tiles=S_tiles):
                        r = k - 4 * j
                        mtrim = 128 * r if r > 0 else 0
                        msl = slice(mtrim, TB)
                        nb_, kc = k // 4, k % 4
                        ksl = slice(128 * kc, 128 * kc + 128)
                        S_t = ps.tile([P, 2, TB], f32, tag="SS",
                                      name=f"S{c}_{j}_{k}", bufs=2)
                        for a in range(2):
                            h = 2 * c + a
                            hsl = slice(32 * h, 32 * h + 32)
                            nc.tensor.matmul(S_t[:, a, msl],
                                             lhsT=k1[nb_][hsl, ksl],
                                             rhs=q1[j][hsl, msl],
                                             start=True, stop=False,
                                             tile_position=(32 * h, 0))
                            nc.tensor.matmul(S_t[:, a, msl],
                                             lhsT=k2[nb_][hsl, ksl],
                                             rhs=q2[j][hsl, msl],
                                             start=False, stop=True,
                                             tile_position=(32 * h, 0))
                        S_tiles[k] = S_t

                    issue_S(0)
                    for k in range(n_k):
                        r = k - 4 * j
                        mtrim = 128 * r if r > 0 else 0
                        msl = slice(mtrim, TB)
                        S_t = S_tiles[k]
                        e_t = tr.tile([P, 2, TB], bf16, tag="eS",
                                      name=f"e{c}_{j}_{k}", bufs=3)
                        nc.scalar.activation(e_t[:, :, msl], S_t[:, :, msl],
                                             Exp, scale=0.125)
                        if r >= 0:
                            tsl = slice(128 * r, 128 * r + 128)
                            nc.gpsimd.tensor_mul(e_t[:, :, tsl], e_t[:, :, tsl],
                                                 mask_r[:])
                        if k + 1 < n_k:
                            issue_S(k + 1)
                        st, sp = (k == 0), (k == n_k - 1)
                        for a in range(2):
                            nc.tensor.matmul(Yh[a][:, msl],
                                             lhsT=v_r[k][:, 2 * c + a, :],
                                             rhs=e_t[:, a, msl],
                                             start=st, stop=sp)
                    for a in range(2):
                        h = 2 * c + a
                        yb = tr.tile([65, TB], f32, tag="cpbuf",
                                     name=f"yb{h}_{j}", bufs=4,
                                     padded_shape=[P, TB])
                        nc.vector.tensor_copy(yb[:], Yh[a][:])
                        nc.sync.dma_start(out=yT32[c][64 * a:64 * a + 64, jsl],
                                          in_=yb[0:64, :])
                        nc.sync.dma_start(out=den4[h:h + 1, jsl],
                                          in_=yb[64:65, :])
                # 1/den (fast approx is plenty for a softmax denominator);
                # bf16 round via bitcast view trick is not possible for
                # matmul rhs, so store bf16 copy inline
                nc.vector.reciprocal_approx_accurate(
                    out=invden4[:, jsl], in_=den4[:, jsl],
                    scratch=recscr[:, jsl])
                nc.vector.tensor_copy(invden_bf[:, jsl], invden4[:, jsl])
                while fillers:          # drain leftovers as a dense burst
                    fillers.pop(0)()
                fillers = make_fillers(j)
            while fillers:
                fillers.pop(0)()

    nc.compile()
    return nc


def _get_module():
    if "nc" not in _CACHE:
        _CACHE["nc"] = _build_module()
        _CACHE["consts"] = _build_consts()
    return _CACHE["nc"], _CACHE["consts"]


def _bf16(a):
    return np.ascontiguousarray(a, dtype=np.float32).astype(ml_dtypes.bfloat16)


def _core_inputs(x, w_q, w_k, w_v, w_proj, core):
    """Build the per-core input map (numpy, host-side sharding)."""
    b = core // 4
    g = core % 4
    heads = [4 * g + j for j in range(HPC)]

    xt = _bf16(np.ascontiguousarray(x[b].T))              # [C, T]

    perm = np.empty(256, dtype=np.int64)
    for m in range(128):
        perm[m] = 64 * heads[m // 32] + (m % 32)             # x1 half
        perm[128 + m] = 64 * heads[m // 32] + 32 + (m % 32)  # x2 half
    wq = _bf16(np.ascontiguousarray(w_q[perm, :].T))         # [C, 256]
    wk = _bf16(np.ascontiguousarray(w_k[perm, :].T))

    vperm = np.empty(256, dtype=np.int64)
    for m in range(256):
        vperm[m] = 64 * heads[m // 64] + (m % 64)
    wv = _bf16(np.ascontiguousarray(w_v[vperm, :].T))        # [C, 256]
    wp = _bf16(np.ascontiguousarray(w_proj[:, vperm].T))     # [256, C]

    return dict(xt=xt, wq=wq, wk=wk, wv=wv, wp=wp)


def kernel(x, w_q, w_k, w_v, w_proj, _trace=False, _trace_cores=None):
    from concourse.bass_utils import run_bass_kernel_spmd

    nc, consts = _get_module()
    x = np.asarray(x, dtype=np.float32)
    in_maps = []
    for core in range(N_CORES):
        m = _core_inputs(np.asarray(x), np.asarray(w_q), np.asarray(w_k),
                         np.asarray(w_v), np.asarray(w_proj), core)
        m.update(consts)
        in_maps.append(m)

    res = run_bass_kernel_spmd(nc, in_maps, list(range(N_CORES)),
                               trace=_trace, trace_cores=_trace_cores)
    outs = [np.asarray(res.results[c]["outT"], dtype=np.float32)
            for c in range(N_CORES)]
    out = np.empty((B, T, C), dtype=np.float32)
    for b in range(B):
        acc = outs[4 * b]
        for g in range(1, 4):
            acc = acc + outs[4 * b + g]
        out[b] = acc.T
    if _trace:
        kernel._last_exec_time_ns = res.exec_time_ns
        kernel._last_results = res
    return out


# revision 63
# speedup vs baseline: 1.0480x; 1.0149x over previous
"""Causal self-attention (RMSNorm-QK + RoPE) Trainium2 Bass kernel.

Problem: B=2, T=2048, C=1024, H=16 heads, D=64.
Sharding: 8 cores = 2 (batch) x 4 (head groups of 4 heads).
Each core computes q/k/v projections for its 4 heads, attention, and a
partial output projection (column-parallel over heads); the host sums the
4 partials per batch and transposes.

All matmuls run in bf16 (inputs rounded on host) with f32 PSUM accumulation.
bf16 halves HBM traffic vs f32r, draws less PE power (less HAM throttle),
and runs full-rate at any free size. Avoid f32r DMA loads entirely: the
f32r-rounding DMA pass truncates mantissas over a wider SBUF region than
its own tile and corrupts bf16 neighbours.

Per-core layouts:
  projection chunks [128, 512]: row 32h+i = head h, rope-half dim i
  q1/q2/k1/k2[n]  [128, 512] bf16 : rope outputs per t-block, kept in the
      32h+i row layout; scores contract rc1+rc2 with two K=32 matmuls per
      head at PE row-group 32h (heads of a pair run concurrently).
  v_r[s]          [128, 4, 65] bf16 : key-chunk s, head h at [:, h, 0:64],
      ones column at [:, h, 64] (softmax denominator trick)
  S (tag SS)      [128, 2, 512] f32 PSUM : scores for one head pair
  yT32[c]         [128, 2048] f32 : heads (2c, 2c+1) attention numerator
  yT_bf[c]        [128, 2048] bf16 : normalized (divided by denominator)
Output: outT [1024, 2048] bf16 = (partial out).T per core; host sums.

Pipelining: block n's RMS+rope tensor/vector work is deferred until block
n+1's projection matmuls are issued (the scalar Ln/Exp latency hides under
them); block j's normalize+out-projection matmuls are spread as fillers
into block j+1's attention stream so the tensor engine never idles long
enough to trip the HAM half-clock throttle.
"""

import sys

for _p in ("/opt/trn_rl_repo",):
    if _p not in sys.path:
        sys.path.append(_p)

import numpy as np
import ml_dtypes

B, T, C = 2, 2048, 1024
H_TOT, D = 16, 64
HPC = 4               # heads per core
N_CORES = 8
P = 128               # partitions
NB = 4                # t-blocks of 512
TB = 512              # t-block size
KCH = 8               # C / 128 contraction chunks
RMS_EPS = 1.1920928955078125e-07
ROPE_BASE = 10000.0

_CACHE = {}


def _build_consts():
    """Host-side constant tensors shared by all cores."""
    inv_freq = (1.0 / (ROPE_BASE ** (np.arange(0, D, 2, dtype=np.float32) / np.float32(D)))).astype(np.float32)
    pos = np.arange(T, dtype=np.float32)
    freqs = np.outer(pos, inv_freq).astype(np.float32)      # [T, 32]
    cos = np.cos(freqs).astype(np.float32)                  # [T, 32]
    sin = np.sin(freqs).astype(np.float32)
    cosr = np.ascontiguousarray(np.tile(cos.T, (HPC, 1))).astype(ml_dtypes.bfloat16)
    sinr = np.ascontiguousarray(np.tile(sin.T, (HPC, 1))).astype(ml_dtypes.bfloat16)
    # ind32 [128, 4]: per-32-row-group summing matrix (lhsT for RMS sums)
    ind32 = np.zeros((P, HPC), dtype=np.float32)
    for p_ in range(P):
        ind32[p_, p_ // 32] = 1.0
    # bc32 [4, 128]: broadcast inv (4 heads) to 32-row groups (lhsT)
    bc32 = np.zeros((HPC, P), dtype=np.float32)
    for p_ in range(P):
        bc32[p_ // 32, p_] = 1.0
    # selpair4 [4, 256]: pair c: out row m <- den row (2c + m//64)
    selpair4 = np.zeros((HPC, 2 * P), dtype=np.float32)
    for c in range(2):
        for m in range(P):
            selpair4[2 * c + m // 64, 128 * c + m] = 1.0
    # causal triangle mask [128, 2, 128] bf16 (same triangle both halves):
    # keep element (p, :, i) iff i >= p
    tri = (np.arange(P)[None, :] >= np.arange(P)[:, None]).astype(np.float32)
    maskt = np.ascontiguousarray(
        np.broadcast_to(tri[:, None, :], (P, 2, P))).astype(ml_dtypes.bfloat16)
    bf = ml_dtypes.bfloat16
    return dict(cosr=cosr, sinr=sinr, ind32=ind32.astype(bf),
                bc32=bc32.astype(bf), selpair4=selpair4.astype(bf),
                maskt=maskt)


def _build_module():
    import concourse.bacc as bacc
    import concourse.mybir as mybir
    import concourse.tile as tile

    f32 = mybir.dt.float32
    bf16 = mybir.dt.bfloat16
    Exp = mybir.ActivationFunctionType.Exp
    Ln = mybir.ActivationFunctionType.Ln
    Copy = mybir.ActivationFunctionType.Copy

    nc = bacc.Bacc("TRN2", target_bir_lowering=False, debug=False,
                   num_devices=N_CORES)

    xt_d = nc.dram_tensor("xt", [C, T], bf16, kind="ExternalInput").ap()
    wq_d = nc.dram_tensor("wq", [C, 256], bf16, kind="ExternalInput").ap()
    wk_d = nc.dram_tensor("wk", [C, 256], bf16, kind="ExternalInput").ap()
    wv_d = nc.dram_tensor("wv", [C, 256], bf16, kind="ExternalInput").ap()
    wp_d = nc.dram_tensor("wp", [256, C], bf16, kind="ExternalInput").ap()
    cosr_d = nc.dram_tensor("cosr", [P, T], bf16, kind="ExternalInput").ap()
    sinr_d = nc.dram_tensor("sinr", [P, T], bf16, kind="ExternalInput").ap()
    ind32_d = nc.dram_tensor("ind32", [P, HPC], bf16, kind="ExternalInput").ap()
    bc32_d = nc.dram_tensor("bc32", [HPC, P], bf16, kind="ExternalInput").ap()
    selpair4_d = nc.dram_tensor("selpair4", [HPC, 2 * P], bf16, kind="ExternalInput").ap()
    maskt_d = nc.dram_tensor("maskt", [P, 2, P], bf16, kind="ExternalInput").ap()
    out_d = nc.dram_tensor("outT", [C, T], bf16, kind="ExternalOutput").ap()

    with tile.TileContext(nc) as tc:
        with (
            tc.tile_pool(name="sb", bufs=1) as sb,
            tc.tile_pool(name="trans", bufs=2) as tr,
            tc.tile_pool(name="ps", bufs=1, space="PSUM") as ps,
        ):
            def direct_load(name, dram_slice, shape, dt):
                t_r = sb.tile(shape, dt, tag=name, name=name)
                nc.sync.dma_start(out=t_r[:], in_=dram_slice)
                return t_r

            # ---- tiny consts first ----
            ind32_r = direct_load("ind32r", ind32_d[:, :], [P, HPC], bf16)
            bc32_r = direct_load("bc32r", bc32_d[:, :], [HPC, P], bf16)
            selpair_r = direct_load("selpairr", selpair4_d[:, :], [HPC, 2 * P], bf16)
            mask_r = direct_load("maskr", maskt_d[:, :, :], [P, 2, P], bf16)

            eps_t = sb.tile([HPC, 1], f32, tag="epst", name="eps_t")
            nc.gpsimd.memset(eps_t[:], RMS_EPS)

            # q/k weight tiles; DMAs issued interleaved with block-0 x below
            wq_r = [sb.tile([P, 256], bf16, tag=f"wqr{k}", name=f"wqr{k}")
                    for k in range(KCH)]
            wk_r = [sb.tile([P, 256], bf16, tag=f"wkr{k}", name=f"wkr{k}")
                    for k in range(KCH)]

            # ---- persistent intermediates ----
            q1 = [sb.tile([P, TB], bf16, tag=f"q1_{n}", name=f"q1_{n}")
                  for n in range(NB)]
            q2 = [sb.tile([P, TB], bf16, tag=f"q2_{n}", name=f"q2_{n}")
                  for n in range(NB)]
            k1 = [sb.tile([P, TB], bf16, tag=f"k1_{n}", name=f"k1_{n}")
                  for n in range(NB)]
            k2 = [sb.tile([P, TB], bf16, tag=f"k2_{n}", name=f"k2_{n}")
                  for n in range(NB)]
            v_r = [sb.tile([P, HPC, 65], bf16, tag=f"v{s}", name=f"v{s}")
                   for s in range(T // P)]
            for s in range(T // P):
                nc.gpsimd.memset(v_r[s][:, :, 64:65], 1.0)
            yT32 = [sb.tile([P, T], f32, tag=f"yT32_{c}", name=f"yT32_{c}")
                    for c in range(2)]
            yT_bf = [sb.tile([P, T], bf16, tag=f"yTbf_{c}", name=f"yTbf_{c}")
                     for c in range(2)]
            den4 = sb.tile([HPC, T], f32, tag="den4", name="den4")
            invden4 = sb.tile([HPC, T], f32, tag="invden4", name="invden4")
            invden_bf = sb.tile([HPC, T], bf16, tag="invdenbf", name="invden_bf")
            recscr = sb.tile([HPC, T], f32, tag="recscr", name="recscr")

            # PSUM tags: SS [128,2,512] x2 bufs (4 banks), YA/YB [65,512]
            # (2 banks), PF [128,512] x2 bufs (2 banks) = 8 banks exactly.

            # ====== Phase 1: projections; RMS+rope deferred one block ======
            deferred = None
            for n in range(NB):
                nsl = slice(n * TB, (n + 1) * TB)
                xr_t = []
                for k in range(KCH):
                    xr = tr.tile([P, TB], bf16, tag="xr", name=f"xr{n}_{k}", bufs=16)
                    if n == 0:
                        nc.sync.dma_start(out=wq_r[k][:],
                                          in_=wq_d[k * P:(k + 1) * P, :])
                        nc.sync.dma_start(out=wk_r[k][:],
                                          in_=wk_d[k * P:(k + 1) * P, :])
                    nc.sync.dma_start(out=xr[:], in_=xt_d[k * P:(k + 1) * P, nsl])
                    xr_t.append(xr)
                if n == 0:
                    # not needed until later: load behind the block-0 stream
                    wv_r = [direct_load(f"wvr{k}", wv_d[k * P:(k + 1) * P, :],
                                        [P, 256], bf16) for k in range(KCH)]
                    cosr_t = direct_load("cosr", cosr_d[:, :], [P, T], bf16)
                    sinr_t = direct_load("sinr", sinr_d[:, :], [P, T], bf16)
                    wp_r = [direct_load(f"wpr{c}", wp_d[c * P:(c + 1) * P, :],
                                        [P, C], bf16) for c in range(2)]
                pq = ps.tile([P, 2, TB], f32, tag="SS", name=f"pq_{n}", bufs=2)
                pk = ps.tile([P, 2, TB], f32, tag="SS", name=f"pk_{n}", bufs=2)
                for k in range(KCH):
                    xr = xr_t[k]
                    st = (k == 0)
                    sp = (k == KCH - 1)
                    nc.tensor.matmul(pq[:, 0, :], lhsT=wq_r[k][:, 0:128], rhs=xr[:],
                                     start=st, stop=sp)
                    nc.tensor.matmul(pq[:, 1, :], lhsT=wq_r[k][:, 128:256], rhs=xr[:],
                                     start=st, stop=sp)
                    nc.tensor.matmul(pk[:, 0, :], lhsT=wk_r[k][:, 0:128], rhs=xr[:],
                                     start=st, stop=sp)
                    nc.tensor.matmul(pk[:, 1, :], lhsT=wk_r[k][:, 128:256], rhs=xr[:],
                                     start=st, stop=sp)
                # fused PSUM -> SBUF copies, bf16 out (2x DVE rate downstream)
                x12q = tr.tile([P, 2, TB], bf16, tag="x12q", name=f"x12q{n}", bufs=1)
                x12k = tr.tile([P, 2, TB], bf16, tag="x12k", name=f"x12k{n}", bufs=1)
                nc.vector.tensor_copy(x12q[:], pq[:])
                nc.vector.tensor_copy(x12k[:], pk[:])
                # v projections (token-major: lhsT = x chunk)
                pv = [ps.tile([P, 256], f32, tag=("YA", "YB")[s % 2],
                              name=f"pv{n}_{s}") for s in range(4)]
                for k in range(KCH):
                    st = (k == 0)
                    sp = (k == KCH - 1)
                    for s_rel in range(4):
                        nc.tensor.matmul(
                            pv[s_rel][:],
                            lhsT=xr_t[k][:, s_rel * P:(s_rel + 1) * P],
                            rhs=wv_r[k][:], start=st, stop=sp)
                for s_rel in range(4):
                    nc.vector.tensor_copy(v_r[4 * n + s_rel][:, :, 0:64],
                                          pv[s_rel][:])
                # RMS sums + Ln/Exp (scalar); rope runs NOW on raw x (it is
                # linear, so the 1/rms scale is applied after, deferred one
                # block so the scalar Ln/Exp latency never stalls the PE)
                invcs = {}
                lns = {}
                for (x12, eng) in ((x12q, "q"), (x12k, "k")):
                    e = nc.vector if eng == "q" else nc.gpsimd
                    ta = "sq" + eng
                    sq12 = tr.tile([P, 2, TB], bf16, tag=ta, name=f"sq{eng}{n}", bufs=1)
                    e.tensor_mul(sq12[:], x12[:], x12[:])
                    ps_s = ps.tile([HPC, TB], f32, tag="PF",
                                   name=f"pss{eng}{n}", bufs=2)
                    nc.tensor.matmul(ps_s[:], lhsT=ind32_r[:], rhs=sq12[:, 0, :],
                                     start=True, stop=False)
                    nc.tensor.matmul(ps_s[:], lhsT=ind32_r[:], rhs=sq12[:, 1, :],
                                     start=False, stop=True)
                    lnm = tr.tile([HPC, TB], f32, tag="lnm" + eng,
                                  name=f"lnm{eng}{n}", bufs=1)
                    lns[eng] = (ps_s, lnm)
                for eng in ("q", "k"):
                    ps_s, lnm = lns[eng]
                    nc.scalar.activation(lnm[:], ps_s[:], Ln,
                                         bias=eps_t[:], scale=1.0 / 64.0)
                for eng in ("q", "k"):
                    ps_s, lnm = lns[eng]
                    invc = tr.tile([HPC, TB], bf16, tag="invc" + eng,
                                   name=f"invc{eng}{n}")
                    nc.scalar.activation(invc[:], lnm[:], Exp, scale=-0.5)
                    invcs[eng] = invc
                # rope immediately (raw, un-normalized), bf16 2x DVE rate
                for (x12, d1, d2, eng) in ((x12q, q1, q2, "q"),
                                           (x12k, k1, k2, "k")):
                    e = nc.vector if eng == "q" else nc.gpsimd
                    ta, tb = (("rtA", "rtB") if eng == "q" else ("rtC", "rtD"))
                    m_a = tr.tile([P, TB], bf16, tag=ta, name=f"ma{eng}{n}", bufs=1)
                    m_b = tr.tile([P, TB], bf16, tag=tb, name=f"mb{eng}{n}", bufs=1)
                    e.tensor_mul(m_a[:], x12[:, 0, :], cosr_t[:, nsl])
                    e.tensor_mul(m_b[:], x12[:, 1, :], sinr_t[:, nsl])
                    e.tensor_add(d1[n][:], m_a[:], m_b[:])
                    m_c = tr.tile([P, TB], bf16, tag=ta, name=f"mc{eng}{n}", bufs=1)
                    m_d = tr.tile([P, TB], bf16, tag=tb, name=f"md{eng}{n}", bufs=1)
                    e.tensor_mul(m_c[:], x12[:, 1, :], cosr_t[:, nsl])
                    e.tensor_mul(m_d[:], x12[:, 0, :], sinr_t[:, nsl])
                    e.tensor_sub(d2[n][:], m_c[:], m_d[:])

                def make_deferred(n=n, invcs=invcs):
                    def run():
                        for (d1, d2, eng) in ((q1, q2, "q"), (k1, k2, "k")):
                            e = nc.vector if eng == "q" else nc.gpsimd
                            ps_b = ps.tile([P, TB], f32, tag="PF",
                                           name=f"psb{eng}{n}", bufs=2)
                            nc.tensor.matmul(ps_b[:], lhsT=bc32_r[:],
                                             rhs=invcs[eng][:],
                                             start=True, stop=True)
                            pb_bf = tr.tile([P, TB], bf16, tag="pbbf" + eng,
                                            name=f"pbbf{eng}{n}", bufs=1)
                            nc.vector.tensor_copy(pb_bf[:], ps_b[:])
                            e.tensor_mul(d1[n][:], d1[n][:], pb_bf[:])
                            e.tensor_mul(d2[n][:], d2[n][:], pb_bf[:])
                    return run

                if deferred is not None:
                    deferred()
                deferred = make_deferred()
            deferred()

            # ====== Phase 2: attention; normalize+out-proj of block j
            # deferred into block j-1's attention stream as per-chunk
            # fillers (keeps the PE stream gapless so HAM stays at full
            # clock), with any remainder as a dense burst at j end ======
            def norm_filler(j, c):
                jsl = slice(j * TB, (j + 1) * TB)

                def run():
                    ps_i = ps.tile([P, TB], f32, tag="PF",
                                   name=f"psi{c}{j}", bufs=2)
                    nc.tensor.matmul(ps_i[:],
                                     lhsT=selpair_r[:, c * P:(c + 1) * P],
                                     rhs=invden_bf[:, jsl],
                                     start=True, stop=True)
                    nc.vector.tensor_mul(yT_bf[c][:, jsl],
                                         yT32[c][:, jsl], ps_i[:])
                return run

            def po_filler(j, o):
                jsl = slice(j * TB, (j + 1) * TB)

                def run():
                    osl = slice(o * P, (o + 1) * P)
                    po = ps.tile([P, TB], f32, tag="PF",
                                 name=f"po{o}_{j}", bufs=2)
                    nc.tensor.matmul(po[:], lhsT=wp_r[0][:, osl],
                                     rhs=yT_bf[0][:, jsl],
                                     start=True, stop=False)
                    nc.tensor.matmul(po[:], lhsT=wp_r[1][:, osl],
                                     rhs=yT_bf[1][:, jsl],
                                     start=False, stop=True)
                    ob = tr.tile([P, TB], bf16, tag="ob",
                                 name=f"ob{o}_{j}", bufs=3)
                    nc.vector.tensor_copy(ob[:], po[:])
                    nc.sync.dma_start(out=out_d[osl, jsl], in_=ob[:])
                return run

            def make_fillers(j):
                return ([norm_filler(j, c) for c in range(2)]
                        + [po_filler(j, o) for o in range(8)])

            # j descending: the dense j=3 attention stream right after P1
            # keeps the PE warm; small-j blocks at the end coincide with the
            # out-projection fillers
            fillers = []
            for j in reversed(range(NB)):
                jsl = slice(j * TB, (j + 1) * TB)
                n_k = 4 * j + 4
                for c in range(2):
                    Yh = [ps.tile([65, TB], f32, tag=("YA", "YB")[a],
                                  name=f"Y{c}_{a}_{j}") for a in range(2)]
                    S_tiles = [None] * n_k

                    def issue_S(k, c=c, j=j, S_tiles=S_tiles):
                        r = k - 4 * j
                        mtrim = 128 * r if r > 0 else 0
                        msl = slice(mtrim, TB)
                        nb_, kc = k // 4, k % 4
                        ksl = slice(128 * kc, 128 * kc + 128)
                        S_t = ps.tile([P, 2, TB], f32, tag="SS",
                                      name=f"S{c}_{j}_{k}", bufs=2)
                        for a in range(2):
                            h = 2 * c + a
                            hsl = slice(32 * h, 32 * h + 32)
                            nc.tensor.matmul(S_t[:, a, msl],
                                             lhsT=k1[nb_][hsl, ksl],
                                             rhs=q1[j][hsl, msl],
                                             start=True, stop=False,
                                             tile_position=(32 * h, 0))
                            nc.tensor.matmul(S_t[:, a, msl],
                                             lhsT=k2[nb_][hsl, ksl],
                                             rhs=q2[j][hsl, msl],
                                             start=False, stop=True,
                                             tile_position=(32 * h, 0))
                        S_tiles[k] = S_t

                    issue_S(0)
                    for k in range(n_k):
                        r = k - 4 * j
                        mtrim = 128 * r if r > 0 else 0
                        msl = slice(mtrim, TB)
                        S_t = S_tiles[k]
                        e_t = tr.tile([P, 2, TB], bf16, tag="eS",
                                      name=f"e{c}_{j}_{k}", bufs=3)
                        nc.scalar.activation(e_t[:, :, msl], S_t[:, :, msl],
                                             Exp, scale=0.125)
                        if r >= 0:
                            tsl = slice(128 * r, 128 * r + 128)
                            nc.vector.tensor_mul(e_t[:, :, tsl], e_t[:, :, tsl],
                                                 mask_r[:])
                        if k + 1 < n_k:
                            issue_S(k + 1)
                        st, sp = (k == 0), (k == n_k - 1)
                        for a in range(2):
                            nc.tensor.matmul(Yh[a][:, msl],
                                             lhsT=v_r[k][:, 2 * c + a, :],
                                             rhs=e_t[:, a, msl],
                                             start=st, stop=sp)
                    for a in range(2):
                        h = 2 * c + a
                        yb = tr.tile([65, TB], f32, tag="cpbuf",
                                     name=f"yb{h}_{j}", bufs=4,
                                     padded_shape=[P, TB])
                        nc.vector.tensor_copy(yb[:], Yh[a][:])
                        nc.sync.dma_start(out=yT32[c][64 * a:64 * a + 64, jsl],
                                          in_=yb[0:64, :])
                        nc.sync.dma_start(out=den4[h:h + 1, jsl],
                                          in_=yb[64:65, :])
                # 1/den (fast approx is plenty for a softmax denominator);
                # bf16 round via bitcast view trick is not possible for
                # matmul rhs, so store bf16 copy inline
                nc.vector.reciprocal_approx_accurate(
                    out=invden4[:, jsl], in_=den4[:, jsl],
                    scratch=recscr[:, jsl])
                nc.vector.tensor_copy(invden_bf[:, jsl], invden4[:, jsl])
                while fillers:          # drain leftovers as a dense burst
                    fillers.pop(0)()
                fillers = make_fillers(j)
            while fillers:
                fillers.pop(0)()

    nc.compile()
    return nc


def _get_module():
    if "nc" not in _CACHE:
        _CACHE["nc"] = _build_module()
        _CACHE["consts"] = _build_consts()
    return _CACHE["nc"], _CACHE["consts"]


def _bf16(a):
    return np.ascontiguousarray(a, dtype=np.float32).astype(ml_dtypes.bfloat16)


def _core_inputs(x, w_q, w_k, w_v, w_proj, core):
    """Build the per-core input map (numpy, host-side sharding)."""
    b = core // 4
    g = core % 4
    heads = [4 * g + j for j in range(HPC)]

    xt = _bf16(np.ascontiguousarray(x[b].T))              # [C, T]

    perm = np.empty(256, dtype=np.int64)
    for m in range(128):
        perm[m] = 64 * heads[m // 32] + (m % 32)             # x1 half
        perm[128 + m] = 64 * heads[m // 32] + 32 + (m % 32)  # x2 half
    wq = _bf16(np.ascontiguousarray(w_q[perm, :].T))         # [C, 256]
    wk = _bf16(np.ascontiguousarray(w_k[perm, :].T))

    vperm = np.empty(256, dtype=np.int64)
    for m in range(256):
        vperm[m] = 64 * heads[m // 64] + (m % 64)
    wv = _bf16(np.ascontiguousarray(w_v[vperm, :].T))        # [C, 256]
    wp = _bf16(np.ascontiguousarray(w_proj[:, vperm].T))     # [256, C]

    return dict(xt=xt, wq=wq, wk=wk, wv=wv, wp=wp)


def kernel(x, w_q, w_k, w_v, w_proj, _trace=False, _trace_cores=None):
    from concourse.bass_utils import run_bass_kernel_spmd

    nc, consts = _get_module()
    x = np.asarray(x, dtype=np.float32)
    in_maps = []
    for core in range(N_CORES):
        m = _core_inputs(np.asarray(x), np.asarray(w_q), np.asarray(w_k),
                         np.asarray(w_v), np.asarray(w_proj), core)
        m.update(consts)
        in_maps.append(m)

    res = run_bass_kernel_spmd(nc, in_maps, list(range(N_CORES)),
                               trace=_trace, trace_cores=_trace_cores)
    outs = [np.asarray(res.results[c]["outT"], dtype=np.float32)
            for c in range(N_CORES)]
    out = np.empty((B, T, C), dtype=np.float32)
    for b in range(B):
        acc = outs[4 * b]
        for g in range(1, 4):
            acc = acc + outs[4 * b + g]
        out[b] = acc.T
    if _trace:
        kernel._last_exec_time_ns = res.exec_time_ns
        kernel._last_results = res
    return out
